# revision 2
# baseline (speedup 1.0000x reference)
"""3x3 neighborhood cosine-similarity sum (minus self) on 8 TRN2 NeuronCores.

v3 design: h-rows on partitions, natural [p, w, c] tiles (fp16 after square/
normalize). Per core: 128 consecutive image rows on the 128 SBUF partitions.

Per w-block (64 interior + 1 halo col each side = 66):
 - DMA   xt [128p, 66w, 128c] f32 (contiguous 33.8KB per partition)
 - ACT   sq = Square(xt) -> fp16
 - DVE   tree-fold c (2x fp16) -> ss;  ACT sqrt(+1e-8);  DVE recip -> sinv
 - DVE   xn = xt * sinv (c-broadcast, 1x) -> fp16
 - PE    S = sum_{dv,dh in 3x3} xn[p+dv, w+dh]: tridiagonal stationary T
         handles vertical (partition) shifts, three moving-AP w-shifts handle
         horizontal; accumulated in PSUM, one matmul per 4w (= 1 PSUM bank)
 - ACT   sf = fp16 copy of S (PSUM -> SBUF)
 - DVE   pr = xn * sf (2x), tree-fold c -> dot;  sim = dot - 1
Shard-edge vertical halo (rows lo-1 / lo+128) handled by a small layout-A
side pipeline computing corr = <xn_edge, hbox3(xn_halo)>, added on the host.
"""

import numpy as np

import sys

for _p in ("/opt/trn_rl_repo",):
    if _p not in sys.path:
        sys.path.insert(0, _p)

import concourse.bass as bass
import concourse.bacc as bacc
import concourse.mybir as mybir
import concourse.tile as tile
from concourse.bass_utils import run_bass_kernel_spmd

F32 = mybir.dt.float32
F16 = mybir.dt.float16
ALU = mybir.AluOpType
ACTF = mybir.ActivationFunctionType
AX = mybir.AxisListType

H, W, C = 1024, 1024, 128
NCORES = 8
RPC = H // NCORES        # 128 rows per core
WB = 64                  # interior w per block
WT = WB + 2              # block tile width incl halo cols
NBLK = W // WB           # 16
SUB = 16                 # w per PSUM tile
BNK = 4                  # w per matmul group (512 f32 = one PSUM bank)
NSUB = WB // SUB         # 4


def build_consts():
    t = np.zeros((128, 128), np.float32)
    for k in range(128):
        for m in (k - 1, k, k + 1):
            if 0 <= m < 128:
                t[k, m] = 1.0
    el = np.zeros((128, 128), np.float32)
    el[127, 0] = 1.0
    er = np.zeros((128, 128), np.float32)
    er[0, 127] = 1.0
    f = lambda a: a.astype(np.float16)
    return f(t), f(el), f(er)


def build_bass():
    nc = bacc.Bacc(None, target_bir_lowering=False)
    x_dram = nc.declare_dram_parameter("x", [RPC, W + 2, C], F32, isOutput=False)
    xm_dram = nc.declare_dram_parameter("xm", [4, W + 2, C], F32, isOutput=False)
    t_dram = nc.declare_dram_parameter("bandt", [128, 128], F16, isOutput=False)
    el_dram = nc.declare_dram_parameter("el", [128, 128], F16, isOutput=False)
    er_dram = nc.declare_dram_parameter("er", [128, 128], F16, isOutput=False)
    sim_dram = nc.declare_dram_parameter("sim", [RPC, W], F32, isOutput=True)
    corr_dram = nc.declare_dram_parameter("corr", [128, 16], F32, isOutput=True)

    with tile.TileContext(nc) as tc:
        with tc.tile_pool(name="consts", bufs=1) as cpool:
            Tt = cpool.tile([128, 128], F16, tag="T")
            el = cpool.tile([128, 128], F16, tag="el")
            er = cpool.tile([128, 128], F16, tag="er")
            nc.sync.dma_start(Tt[:], t_dram[:])
            nc.sync.dma_start(el[:], el_dram[:])
            nc.sync.dma_start(er[:], er_dram[:])
            # eps on ss (=norm^2): sinv <= 1/sqrt(1e-8) = 1e4 stays fp16-finite
            eps = cpool.tile([128, 1], F32, tag="eps")
            nc.gpsimd.memset(eps[:], 1e-8)
            dfull = cpool.tile([128, W], F32, tag="dfull")

            # ---------- main pipeline
            with (
                tc.tile_pool(name="xt", bufs=2) as xtpool,
                tc.tile_pool(name="sq", bufs=2) as sqpool,
                tc.tile_pool(name="tr1", bufs=2) as tr1pool,
                tc.tile_pool(name="tr2", bufs=2) as tr2pool,
                tc.tile_pool(name="ssn", bufs=2) as ssnpool,
                tc.tile_pool(name="sih", bufs=2) as sihpool,
                tc.tile_pool(name="xn", bufs=2) as xnpool,
                tc.tile_pool(name="sf", bufs=2) as sfpool,
                tc.tile_pool(name="pr", bufs=1) as prpool,
                tc.tile_pool(name="psum", bufs=2, space="PSUM") as psumpool,
            ):
                def phase1(i):
                    """DMA + square + norm tree + sinv + xn for block i."""
                    xt = xtpool.tile([128, WT, C], F32, tag="xt")
                    nc.sync.dma_start(xt[:], x_dram[:, WB * i : WB * i + WT, :])
                    sq = sqpool.tile([128, WT, C], F16, tag="sq")
                    nc.scalar.activation(sq[:], xt[:], ACTF.Square)
                    t1 = tr1pool.tile([128, WT, 64], F16, tag="t1")
                    t2 = tr2pool.tile([128, WT, 32], F16, tag="t2")
                    ss = ssnpool.tile([128, WT, 1], F32, tag="ss")
                    nc.vector.tensor_add(t1[:], sq[:, :, 0:64], sq[:, :, 64:128])
                    nc.vector.tensor_add(t2[:], t1[:, :, 0:32], t1[:, :, 32:64])
                    nc.vector.tensor_add(
                        t1[:, :, 0:16], t2[:, :, 0:16], t2[:, :, 16:32]
                    )
                    nc.vector.tensor_add(
                        t2[:, :, 0:8], t1[:, :, 0:8], t1[:, :, 8:16]
                    )
                    nc.vector.tensor_reduce(
                        ss[:, :, 0], t2[:, :, 0:8], axis=AX.X, op=ALU.add
                    )
                    nrm = ssnpool.tile([128, WT, 1], F32, tag="nrm")
                    nc.scalar.activation(nrm[:], ss[:], ACTF.Sqrt, bias=eps[:])
                    sv = ssnpool.tile([128, WT, 1], F32, tag="sv")
                    nc.vector.reciprocal(sv[:], nrm[:])
                    sih = sihpool.tile([128, WT, 1], F16, tag="sih")
                    nc.vector.tensor_copy(sih[:], sv[:])
                    xn = xnpool.tile([128, WT, C], F16, tag="xn")
                    xn_eng = nc.gpsimd if i % 3 == 2 else nc.vector
                    xn_eng.tensor_tensor(
                        xn[:], xt[:], sih[:].broadcast_to([128, WT, C]), ALU.mult
                    )
                    return xn

                def phase2(i, xn):
                    """Box matmuls + evac + dot for block i."""
                    pr = prpool.tile([128, WB, C], F16, tag="pr")
                    for s in range(NSUB):
                        S = psumpool.tile([128, SUB, C], F32, tag="S")
                        b = 1 + s * SUB
                        # matmul out <= 512 f32 (one PSUM bank): 4w per group
                        for wg in range(SUB // BNK):
                            wa = wg * BNK
                            for di, dlt in enumerate((-1, 0, 1)):
                                nc.tensor.matmul(
                                    S[:, wa : wa + BNK, :],
                                    Tt[:],
                                    xn[:, b + dlt + wa : b + dlt + wa + BNK, :],
                                    start=(di == 0),
                                    stop=(di == 2),
                                )
                        sf = sfpool.tile([128, SUB, C], F16, tag="sf")
                        nc.scalar.activation(sf[:], S[:], ACTF.Copy)
                        nc.vector.tensor_tensor(
                            pr[:, s * SUB : (s + 1) * SUB, :],
                            xn[:, b : b + SUB, :],
                            sf[:],
                            ALU.mult,
                        )
                    # dot tree over c -> dfull[:, 64i:64i+64]
                    d1 = tr1pool.tile([128, WT, 64], F16, tag="t1")
                    d2 = tr2pool.tile([128, WT, 32], F16, tag="t2")
                    nc.vector.tensor_add(
                        d1[:, 0:WB, :], pr[:, :, 0:64], pr[:, :, 64:128]
                    )
                    nc.vector.tensor_add(
                        d2[:, 0:WB, :], d1[:, 0:WB, 0:32], d1[:, 0:WB, 32:64]
                    )
                    nc.vector.tensor_add(
                        d1[:, 0:WB, 0:16], d2[:, 0:WB, 0:16], d2[:, 0:WB, 16:32]
                    )
                    nc.vector.tensor_add(
                        d2[:, 0:WB, 0:8], d1[:, 0:WB, 0:8], d1[:, 0:WB, 8:16]
                    )
                    nc.vector.tensor_reduce(
                        dfull[:, WB * i : WB * i + WB],
                        d2[:, 0:WB, 0:8], axis=AX.X, op=ALU.add,
                    )

                # ---------- mini pipeline (shard-edge corrections), split in
                # two and interleaved with the main loop so it overlaps
                def mini_p1():
                    xm4 = xtpool.tile([128, 4, 8, C], F32, tag="xt")
                    for r in range(4):
                        nc.sync.dma_start(
                            xm4[:, r],
                            xm_dram[r, 1 : W + 1, :].rearrange(
                                "(j p) c -> p j c", p=128
                            ),
                        )
                    sqm = tr1pool.tile([128, 4, 8, C], F16, tag="t1")
                    nc.gpsimd.tensor_tensor(sqm[:], xm4[:], xm4[:], ALU.mult)
                    ssm = ssnpool.tile([128, 4, 8], F32, tag="ssm", bufs=1)
                    nc.vector.tensor_reduce(ssm[:], sqm[:], axis=AX.X, op=ALU.add)
                    nrmm = ssnpool.tile([128, 4, 8], F32, tag="nrmm", bufs=1)
                    nc.scalar.activation(nrmm[:], ssm[:], ACTF.Sqrt, bias=eps[:])
                    sinvm = ssnpool.tile([128, 4, 8], F32, tag="sinvm", bufs=1)
                    nc.vector.reciprocal(sinvm[:], nrmm[:])
                    sinvmh = sihpool.tile([128, 4, 8, 1], F16, tag="sinvmh", bufs=1)
                    nc.vector.tensor_copy(sinvmh[:, :, :, 0], sinvm[:])
                    xnm = xnpool.tile([128, 4, 8, C], F16, tag="xnm", bufs=1)
                    nc.gpsimd.tensor_tensor(
                        xnm[:], xm4[:],
                        sinvmh[:].broadcast_to([128, 4, 8, C]), ALU.mult,
                    )
                    return xnm

                def mini_p2(xnm):
                    Sm = psumpool.tile([128, 2, 8, C], F32, tag="S")
                    for gi, r in enumerate((0, 3)):
                        nc.tensor.matmul(
                            Sm[:, gi, 0:4, :], Tt[:], xnm[:, r, 0:4, :],
                            start=True, stop=False,
                        )
                        nc.tensor.matmul(
                            Sm[:, gi, 4:8, :], Tt[:], xnm[:, r, 4:8, :],
                            start=True, stop=False,
                        )
                        nc.tensor.matmul(
                            Sm[:, gi, 1:4, :], el[:], xnm[:, r, 0:3, :],
                            start=False, stop=False,
                        )
                        nc.tensor.matmul(
                            Sm[:, gi, 4:8, :], el[:], xnm[:, r, 3:7, :],
                            start=False, stop=False,
                        )
                        nc.tensor.matmul(
                            Sm[:, gi, 0:4, :], er[:], xnm[:, r, 1:5, :],
                            start=False, stop=True,
                        )
                        nc.tensor.matmul(
                            Sm[:, gi, 4:7, :], er[:], xnm[:, r, 5:8, :],
                            start=False, stop=True,
                        )
                    sfm = sfpool.tile([128, 2, 8, C], F16, tag="sfm", bufs=1)
                    nc.scalar.activation(sfm[:], Sm[:], ACTF.Copy)
                    prm = tr2pool.tile([128, 2, 8, C], F16, tag="t2")
                    nc.vector.tensor_tensor(prm[:], xnm[:, 1:3], sfm[:], ALU.mult)
                    corrt = ssnpool.tile([128, 2, 8], F32, tag="corrt", bufs=1)
                    nc.vector.tensor_reduce(corrt[:], prm[:], axis=AX.X, op=ALU.add)
                    nc.sync.dma_start(
                        corr_dram[:], corrt[:].rearrange("p r j -> p (r j)")
                    )

                # software pipeline: phase1 runs one block ahead of phase2 so
                # ACT's square(i+1) is not queued behind block i's evacs
                xn_prev = phase1(0)
                for i in range(NBLK):
                    xn_next = phase1(i + 1) if i + 1 < NBLK else None
                    phase2(i, xn_prev)
                    xn_prev = xn_next
                xnm = mini_p1()
                mini_p2(xnm)

                # ---- finals: sim = dot - 1 (xn already normalized)
                nc.vector.tensor_scalar(dfull[:], dfull[:], -1.0, None, ALU.add)
                nc.sync.dma_start(sim_dram[:], dfull[:])

    nc.compile()
    return nc


def shard_inputs(input_image):
    x = np.asarray(input_image).reshape(H, W, C).astype(np.float32, copy=False)
    xp = np.zeros((H, W + 2, C), np.float32)
    xp[:, 1 : W + 1] = x
    Tt, el, er = build_consts()
    in_maps = []
    for core in range(NCORES):
        lo = core * RPC
        shard = np.ascontiguousarray(xp[lo : lo + RPC])
        xm = np.zeros((4, W + 2, C), np.float32)
        if lo - 1 >= 0:
            xm[0] = xp[lo - 1]
        xm[1] = xp[lo]
        xm[2] = xp[lo + RPC - 1]
        if lo + RPC < H:
            xm[3] = xp[lo + RPC]
        in_maps.append(
            {"x": shard, "xm": xm, "bandt": Tt, "el": el, "er": er}
        )
    return in_maps


def unshard_output(results):
    out = np.empty((H, W), np.float32)
    for core in range(NCORES):
        lo = core * RPC
        sim = np.asarray(results[core]["sim"]).copy()
        corr = np.asarray(results[core]["corr"]).reshape(128, 2, 8)
        sim[0] += corr[:, 0, :].T.reshape(W)
        sim[RPC - 1] += corr[:, 1, :].T.reshape(W)
        out[lo : lo + RPC] = sim
    return out


_NC_CACHE = {}


def get_nc():
    if "nc" not in _NC_CACHE:
        _NC_CACHE["nc"] = build_bass()
    return _NC_CACHE["nc"]


def kernel(input_image):
    nc = get_nc()
    in_maps = shard_inputs(input_image)
    res = run_bass_kernel_spmd(nc, in_maps, list(range(NCORES)))
    return unshard_output(res.results)


if __name__ == "__main__":
    if "--compile-only" in sys.argv:
        build_bass()
        print("compile OK")
    else:
        rng = np.random.default_rng(0)
        x = rng.standard_normal((H, W, 1, C), dtype=np.float32)
        out = kernel(x)
        print(out.shape, out.dtype, out[:2, :4])
